# revision 35
# baseline (speedup 1.0000x reference)
"""Causal self-attention kernel for Trainium2, sharded over 8 NeuronCores.

Problem: B=4, T=2048, DIM=1024, H=16 heads, head_dim=64, fp32 I/O.

Sharding: (batch, head-group) pairs -> 8 shards. Core c handles batch
b = c//2 and head group g = c%2 (8 heads each). Each core computes its
q/k/v projections for its head slice, causal flash-style attention, and
a partial o_proj against its head-slice of wo. The host sums the two
partial o_proj outputs per batch (the "all-reduce") while gathering.

Schedule strategy (per core): the attention inner loop is ScalarE-bound
(exp runs at 1.2 GHz x 1 elem/lane; the PE needs only ~60% of that
time for scores+AV), so all projection/o_proj matmuls are emitted as a
work queue of half-fills interleaved into the attention j-loop. The
Tile scheduler then keeps the PE dense (no HAM re-throttle) while ACT
streams exps. Heads run one at a time (not in pairs): that halves the
live PSUM set (scores 2x2 banks double-buffered + AV 2 banks) and
leaves 2 banks for the projection queue's accumulators.

Layout (per core):
  - Host pre-transposes x and the weight slices so the contraction dim
    lands on SBUF partitions, and casts to bf16.
  - Scores are computed TRANSPOSED: sT[tk, tq] = k @ q^T, so softmax'd
    probabilities come out with tk on partitions -- the layout the
    attn@v matmul needs as its moving operand (lhsT = v).
  - Softmax skips max-subtraction (scores are O(1) by construction),
    exp runs on ScalarE straight out of PSUM, and the denominator is
    free via a ones column appended to v.
  - 1/denominator runs on the (otherwise idle) DVE via the fast-approx
    Newton reciprocal, broadcast on GpSimd -- no ACT work at all, so
    ScalarE does nothing but exp.
  - Causal masking inside diagonal 128-tiles: one DVE multiply with a
    0/1 lower-triangle mask.
"""

import numpy as np
import ml_dtypes

import concourse.bass as bass
import concourse.bacc as bacc
import concourse.mybir as mybir
import concourse.tile as tile
from concourse.bass import ds, ts
from concourse.bass_utils import run_bass_kernel_spmd

BF16 = mybir.dt.bfloat16
F32 = mybir.dt.float32

T = 2048
D = 1024
DG = 512          # head-group width (8 heads x 64)
NH = 8            # heads per core
DH = 64
P = 128
NT = T // P       # 16 t-tiles
NKO = D // P      # 8 contraction tiles for projections
NC_CHUNK = 1024   # tq chunk width for attention
NCH = T // NC_CHUNK  # 2 chunks

_CACHED = None  # (nc, input names) -- build/trace once per process

MM_N = 512  # max moving free-dim per matmul instruction (one PSUM bank)


def _build_kernel():
    nc = bacc.Bacc("TRN2", target_bir_lowering=False, debug=False)

    xT_d = nc.dram_tensor("xT", [D, T], BF16, kind="ExternalInput").ap()
    wqT_d = nc.dram_tensor("wqT", [D, DG], BF16, kind="ExternalInput").ap()
    wkT_d = nc.dram_tensor("wkT", [D, DG], BF16, kind="ExternalInput").ap()
    wvT_d = nc.dram_tensor("wvT", [D, DG], BF16, kind="ExternalInput").ap()
    woT_d = nc.dram_tensor("woT", [DG, D], BF16, kind="ExternalInput").ap()
    y_d = nc.dram_tensor("y", [T, D], F32, kind="ExternalOutput").ap()

    with tile.TileContext(nc) as tc:
        with (
            tc.tile_pool(name="const", bufs=1) as const,
            tc.tile_pool(name="sb", bufs=1) as sb,
            tc.tile_pool(name="work", bufs=4) as work,
            tc.tile_pool(name="wnorm", bufs=2) as wnorm,
            tc.tile_pool(name="sc", bufs=2, space="PSUM") as scp,
            tc.tile_pool(name="av", bufs=1, space="PSUM") as avp,
            tc.tile_pool(name="pj", bufs=2, space="PSUM") as pjp,
        ):
            # ---- constants ----
            # multiplicative causal mask for diag tiles: 1 where tq >= tk
            mskb = const.tile([P, P], BF16, tag="mskb")
            nc.gpsimd.memset(mskb, 1.0)
            nc.gpsimd.affine_select(
                out=mskb, in_=mskb,
                compare_op=mybir.AluOpType.is_ge,
                fill=0.0, base=0,
                pattern=[[1, P]], channel_multiplier=-1,
            )

            # ---- persistent SBUF tensors ----
            XT = sb.tile([P, NKO, T], BF16, tag="XT")
            WQT = sb.tile([P, NKO, DG], BF16, tag="WQT")
            WKT = sb.tile([P, NKO, DG], BF16, tag="WKT")
            WVT = sb.tile([P, NKO, DG], BF16, tag="WVT")
            WOT = sb.tile([P, DG // P, D], BF16, tag="WOT")
            QT = sb.tile([P, DG // P, T], BF16, tag="QT")
            # zero-padded K copies: KZ0 holds even heads' k rows on
            # partitions 0:64 (zeros above), KZ1 odd heads' on 64:128
            # (zeros below). Scores then contract over the full 128
            # partitions -- the zero half contributes exactly 0 -- so
            # every matmul in the kernel runs in 128x128 tiling mode and
            # the PE never pays a mode-switch drain.
            KZ0 = sb.tile([P, DG // P, T], BF16, tag="KZ0")
            KZ1 = sb.tile([P, DG // P, T], BF16, tag="KZ1")
            VA = sb.tile([P, NT, NH, DH + 1], BF16, tag="VA")
            OGT = sb.tile([P, DG // P, T], BF16, tag="OGT")

            # ---- input DMAs. dma_start issue costs ~0.6us of the issuing
            # engine, so keep the count low and split across the two HWDGE
            # engines (SyncE for x, the startup-idle ScalarE queue for
            # weights). A fine-grained first wave (x tq 0:512 per k, q/k
            # weight dg-0 slices) unblocks the first projection fills
            # megabytes earlier than whole-tensor transfers would ----
            xr = xT_d.rearrange("(ko p) t -> p ko t", p=P)
            wqr = wqT_d.rearrange("(ko p) n -> p ko n", p=P)
            wkr = wkT_d.rearrange("(ko p) n -> p ko n", p=P)
            wvr = wvT_d.rearrange("(ko p) n -> p ko n", p=P)
            wor = woT_d.rearrange("(jo p) n -> p jo n", p=P)
            nc.scalar.dma_start(WQT[:, :, 0:P], wqr[:, :, 0:P])
            nc.scalar.dma_start(WKT[:, :, 0:P], wkr[:, :, 0:P])
            # wv per-k 2D transfers: contiguous 1KB partition rows make
            # descriptor-friendly DMAs (the whole-tensor 3D form lands
            # ~10us later and gates the first AV matmuls)
            for k in range(NKO):
                nc.scalar.dma_start(WVT[:, k, :], wvr[:, k, :])
            for k in range(NKO):
                nc.scalar.dma_start(
                    WQT[:, k, ds(P, DG - P)], wqr[:, k, ds(P, DG - P)])
            for k in range(NKO):
                nc.scalar.dma_start(
                    WKT[:, k, ds(P, DG - P)], wkr[:, k, ds(P, DG - P)])
            for k in range(NKO):
                nc.sync.dma_start(XT[:, k, 0:MM_N], xr[:, k, 0:MM_N])
            for k in range(NKO):
                nc.sync.dma_start(
                    XT[:, k, ds(MM_N, T - MM_N)], xr[:, k, ds(MM_N, T - MM_N)],
                )
            # wo is needed only by the late o_proj fills; per-jo 2D form
            # on the x queue keeps it off the weight queue's critical path
            for j in range(DG // P):
                nc.sync.dma_start(WOT[:, j, :], wor[:, j, :])

            # v_aug ones column
            nc.gpsimd.memset(VA[:, :, :, DH], 1.0)
            # zero halves of the padded K copies, finest-dep-first so the
            # first heads' scores aren't gated on the whole fill
            for dg in range(DG // P):
                nc.gpsimd.memset(KZ0[DH:P, dg, :], 0.0)
                nc.gpsimd.memset(KZ1[0:DH, dg, :], 0.0)

            # ---- projection work queue (half-fill closures) ----
            # Each fill accumulates 1 PSUM bank over its contraction and
            # is split into two emission halves so PE bursts stay under
            # ~1us and never starve ACT of its next scores tile.
            def make_qk_fill(wsb, dst, dsts, dg, tcn):
                st = {}

                def h1():
                    ps = pjp.tile([P, MM_N], F32, tag="pj")
                    st["ps"] = ps
                    for k in range(4):
                        nc.tensor.matmul(
                            ps, lhsT=wsb[:, k, ts(dg, P)],
                            rhs=XT[:, k, ds(tcn * MM_N, MM_N)],
                            start=(k == 0), stop=False,
                        )

                def h2():
                    ps = st["ps"]
                    for k in range(4, NKO):
                        nc.tensor.matmul(
                            ps, lhsT=wsb[:, k, ts(dg, P)],
                            rhs=XT[:, k, ds(tcn * MM_N, MM_N)],
                            start=False, stop=(k == NKO - 1),
                        )
                    sl = ds(tcn * MM_N, MM_N)
                    if dsts is None:
                        nc.vector.tensor_copy(dst[:, dg, sl], ps)
                    else:
                        # k goes into the zero-padded pair, lane-aligned
                        nc.vector.tensor_copy(dst[0:DH, dg, sl], ps[0:DH, :])
                        nc.vector.tensor_copy(dsts[DH:P, dg, sl], ps[DH:P, :])

                return [h1, h2]

            def make_v_fill(tt):
                st = {}

                def h1():
                    ps = pjp.tile([P, MM_N], F32, tag="pj")
                    st["ps"] = ps
                    for k in range(4):
                        nc.tensor.matmul(
                            ps, lhsT=XT[:, k, ts(tt, P)], rhs=WVT[:, k, :],
                            start=(k == 0), stop=False,
                        )

                def h2():
                    ps = st["ps"]
                    for k in range(4, NKO):
                        nc.tensor.matmul(
                            ps, lhsT=XT[:, k, ts(tt, P)], rhs=WVT[:, k, :],
                            start=False, stop=(k == NKO - 1),
                        )
                    nc.vector.tensor_copy(
                        VA[:, tt, :, 0:DH],
                        ps.rearrange("p (h d) -> p h d", h=NH),
                    )

                return [h1, h2]

            def make_oproj_fill(tt, half, pool=None, fast_evac=False):
                st = {}

                def h1():
                    pl = pool if pool is not None else pjp
                    ps = pl.tile([P, MM_N], F32,
                                 tag="pj" if pl is pjp else "sc")
                    st["ps"] = ps
                    for jt in range(2):
                        nc.tensor.matmul(
                            ps, lhsT=OGT[:, jt, ts(tt, P)],
                            rhs=WOT[:, jt, ds(half * MM_N, MM_N)],
                            start=(jt == 0), stop=False,
                        )

                def h2():
                    ps = st["ps"]
                    for jt in range(2, DG // P):
                        nc.tensor.matmul(
                            ps, lhsT=OGT[:, jt, ts(tt, P)],
                            rhs=WOT[:, jt, ds(half * MM_N, MM_N)],
                            start=False, stop=(jt == DG // P - 1),
                        )
                    if fast_evac:
                        # pure-PE tail: PSUM slots gate on these copies,
                        # and ScalarE is idle once the exps are done --
                        # alternate engines so two evacuations overlap,
                        # with 4 staging buffers to decouple the DMAs
                        ysb = work.tile([P, MM_N], F32, tag="ysb2")
                        if (tt + half) % 2:
                            nc.scalar.copy(ysb, ps)
                        else:
                            nc.vector.tensor_copy(ysb, ps)
                    else:
                        ysb = wnorm.tile([P, MM_N], F32, tag="ysb")
                        nc.vector.tensor_copy(ysb, ps)
                    nc.sync.dma_start(y_d[ts(tt, P), ds(half * MM_N, MM_N)], ysb)

                return [h1, h2]

            # Queue order: everything chunk-0 attention needs first (all
            # pairs' q/k for tq 0:1024, v tiles 0..7), then the chunk-1
            # prerequisites. o_proj halves are appended between sweeps.
            # fill_end[key] = queue index at which that tensor region is
            # fully emitted, so the attention loop can pull exactly its
            # prerequisites and otherwise drain at a steady 1 half per j.
            fills = []
            fill_end = {}

            def add(key, halves):
                fills.extend(halves)
                fill_end[key] = len(fills)

            add(("q", 0, 0), make_qk_fill(WQT, QT, None, 0, 0))
            add(("k", 0, 0), make_qk_fill(WKT, KZ0, KZ1, 0, 0))
            add(("q", 0, 1), make_qk_fill(WQT, QT, None, 0, 1))
            add(("v", 0), make_v_fill(0))
            add(("k", 0, 1), make_qk_fill(WKT, KZ0, KZ1, 0, 1))
            add(("v", 1), make_v_fill(1))
            for tt in range(2, 8):
                add(("v", tt), make_v_fill(tt))
            for dg in range(1, 4):
                for tcn in range(2):
                    add(("q", dg, tcn), make_qk_fill(WQT, QT, None, dg, tcn))
                    add(("k", dg, tcn), make_qk_fill(WKT, KZ0, KZ1, dg, tcn))
            for tcn in range(2, 4):
                add(("q", 0, tcn), make_qk_fill(WQT, QT, None, 0, tcn))
                add(("k", 0, tcn), make_qk_fill(WKT, KZ0, KZ1, 0, tcn))
            for tt in range(8, NT):
                add(("v", tt), make_v_fill(tt))
            for dg in range(1, 4):
                for tcn in range(2, 4):
                    add(("q", dg, tcn), make_qk_fill(WQT, QT, None, dg, tcn))
                    add(("k", dg, tcn), make_qk_fill(WKT, KZ0, KZ1, dg, tcn))

            state = {"fi": 0, "hold": 0}

            def pop_until(idx):
                while state["fi"] < idx:
                    fills[state["fi"]]()
                    state["fi"] += 1

            def pop(n=1):
                # paced draining respects the hold-back reservation (work
                # kept for the final head's window); need() ignores it
                pop_until(min(state["fi"] + n, len(fills) - state["hold"]))

            def need(keys):
                pop_until(max(fill_end[k] for k in keys))

            # ---- attention: chunk-major sweep over heads ----
            def attn_head_chunk(h, c, pops=1):
                hp, sub = divmod(h, 2)
                kz = KZ0 if sub == 0 else KZ1
                av = avp.tile([DH + 1, NC_CHUNK], F32, tag="av")
                jmax = (c + 1) * NC_CHUNK // P - 1
                for j in range(jmax + 1):
                    js = [j]
                    req = [("q", hp, tcn)
                           for tcn in range(max(c * NC_CHUNK, j * P) // MM_N,
                                            2 * (c + 1))]
                    req += [("k", hp, j * P // MM_N), ("v", j)]
                    need(req)
                    pop(pops)
                    lo = max(c * NC_CHUNK, j * P)
                    w = (c + 1) * NC_CHUNK - lo
                    et = work.tile([P, NC_CHUNK], BF16, tag="et")
                    if c == 1 and j == 0:
                        # head-boundary de-stall: both "sc" slots are still
                        # pinned by the previous head's last two exp reads,
                        # so route this head's first scores through the pj
                        # pool (idle once the fill queue drains) as two
                        # 1-bank tiles; costs one extra exp call on the
                        # slack-rich ScalarE.
                        for half in range(2):
                            php = pjp.tile([P, MM_N], F32, tag="pj")
                            nc.tensor.matmul(
                                php,
                                lhsT=kz[:, hp, ts(j, P)],
                                rhs=QT[:, hp, ds(lo + half * MM_N, MM_N)],
                                start=True, stop=True,
                            )
                            nc.scalar.activation(
                                et[:, ds(half * MM_N, MM_N)], php,
                                mybir.ActivationFunctionType.Exp,
                                scale=0.125,
                            )
                        ps = None
                    else:
                        ps = scp.tile([P, NC_CHUNK], F32, tag="sc")
                        o = 0
                        while o < w:
                            ww = min(w - o, MM_N)
                            nc.tensor.matmul(
                                ps[:, ds(o, ww)],
                                lhsT=kz[:, hp, ts(j, P)],
                                rhs=QT[:, hp, ds(lo + o, ww)],
                                start=True, stop=True,
                            )
                            o += ww
                    for j, lo, w, ps, _ in [(j, lo, w, ps, None)]:
                        if ps is not None:
                            nc.scalar.activation(
                                et[:, :w], ps[:, :w],
                                mybir.ActivationFunctionType.Exp,
                                scale=0.125,
                            )
                        if j * P >= c * NC_CHUNK:
                            # zero below the diagonal of the diag 128-block
                            # (DVE is idle; keeps masking off ACT/PE)
                            nc.vector.tensor_mul(et[:, 0:P], et[:, 0:P], mskb)
                        # AV accumulate, per psum bank: bank b of this chunk
                        # has its last contribution at j == 8c + 4b + 3.
                        s0 = lo - c * NC_CHUNK
                        for b in range(NC_CHUNK // MM_N):
                            blo, bhi = b * MM_N, (b + 1) * MM_N
                            plo, phi = max(s0, blo), min(s0 + w, bhi)
                            if plo >= phi:
                                continue
                            nc.tensor.matmul(
                                av[0:DH + 1, ds(plo, phi - plo)],
                                lhsT=VA[:, j, h, :],
                                rhs=et[:, ds(plo - s0, phi - plo)],
                                start=(j == 0),
                                stop=(j == 8 * c + 4 * b + 3),
                            )
                    # normalize: copy the accumulator out of PSUM, DMA the
                    # denominator row to partition 0 (engines can't shift
                    # partitions), 1/d on DVE (fast-approx Newton, ~18
                    # bits -- plenty ahead of a bf16 multiply), broadcast
                    # on GpSimd, multiply on DVE. Entirely off-ACT. For
                    # the final head the banks normalize as soon as each
                    # accumulation closes, so the o_proj tail starts ~4
                    # j-iterations earlier; elsewhere both banks go
                    # together after the chunk (fewer ops in the steady
                    # state).
                    last = h == NH - 1 and c == NCH - 1
                    for b in range(NC_CHUNK // MM_N):
                        done_j = 8 * c + 4 * b + 3
                        if last:
                            if done_j not in js:
                                continue
                        elif (jmax not in js) or done_j > jmax:
                            continue
                        sl = ds(b * MM_N, MM_N)
                        osl = ds(c * NC_CHUNK + b * MM_N, MM_N)
                        un = wnorm.tile([DH + 1, MM_N], F32, tag="un")
                        nc.vector.tensor_copy(un, av[0:DH + 1, sl])
                        dr = wnorm.tile([1, MM_N], F32, tag="dr")
                        if last:
                            # ScalarE is idle once the exps end and its
                            # ACTIVATE path can shift base partitions --
                            # cheaper than the DMA round trip on the
                            # critical o_proj-tail chain
                            nc.scalar.copy(dr, un[DH:DH + 1, :])
                        else:
                            nc.sync.dma_start(dr, un[DH:DH + 1, :])
                        rc = wnorm.tile([1, MM_N], F32, tag="dr")
                        nc.vector.reciprocal_approx_fast(rc, dr)
                        rb = wnorm.tile([DH, MM_N], F32, tag="rb")
                        nc.gpsimd.partition_broadcast(rb, rc)
                        if sub == 0:
                            nc.vector.tensor_mul(
                                OGT[0:DH, hp, osl], un[0:DH, :], rb,
                            )
                        else:
                            # DVE can't shift partitions; stage + DMA up
                            stg = wnorm.tile([DH, MM_N], BF16, tag="stg")
                            nc.vector.tensor_mul(stg, un[0:DH, :], rb)
                            nc.sync.dma_start(OGT[DH:P, hp, osl], stg)

            for h in range(NH):
                attn_head_chunk(h, 0)
            # chunk-0 OGT complete -> first-half o_proj can interleave
            # with the chunk-1 sweep; reserve a slice of it for the last
            # head's window so the PE stays fed while the queue runs dry
            for tt in range(8):
                for half in range(2):
                    add(("o", tt, half), make_oproj_fill(tt, half))
            state["hold"] = 12
            for h in range(NH):
                if h == NH - 1:
                    state["hold"] = 0
                attn_head_chunk(h, 1)
            pop_until(len(fills))  # drain any unpopped queue work
            # pure-PE tail: alternate PSUM pools (the score pool is idle
            # now) so four o_proj accumulations pipeline
            for tt in range(8, NT):
                for half in range(2):
                    add(("o", tt, half),
                        make_oproj_fill(tt, half, scp if (tt + half) % 2 else pjp,
                                        fast_evac=True))
            pop_until(len(fills))

    nc.compile()
    return nc


def _get_nc():
    global _CACHED
    if _CACHED is None:
        _CACHED = _build_kernel()
    return _CACHED


def _shard_inputs(x, wq, wk, wv, wo):
    bf = ml_dtypes.bfloat16
    in_maps = []
    for core in range(8):
        b, g = divmod(core, 2)
        gs = slice(g * DG, (g + 1) * DG)
        in_maps.append({
            "xT": np.ascontiguousarray(x[b].T).astype(bf),
            "wqT": np.ascontiguousarray(wq[gs, :].T).astype(bf),
            "wkT": np.ascontiguousarray(wk[gs, :].T).astype(bf),
            "wvT": np.ascontiguousarray(wv[gs, :].T).astype(bf),
            "woT": np.ascontiguousarray(wo[:, gs].T).astype(bf),
        })
    return in_maps


def kernel(x, wq, wk, wv, wo, _trace=False, _trace_cores=None):
    x = np.asarray(x, dtype=np.float32)
    wq = np.asarray(wq, dtype=np.float32)
    wk = np.asarray(wk, dtype=np.float32)
    wv = np.asarray(wv, dtype=np.float32)
    wo = np.asarray(wo, dtype=np.float32)

    nc = _get_nc()
    in_maps = _shard_inputs(x, wq, wk, wv, wo)
    res = run_bass_kernel_spmd(
        nc, in_maps, core_ids=list(range(8)),
        trace=_trace,
        **({"trace_cores": _trace_cores} if _trace_cores else {}),
    )
    B = x.shape[0]
    y = np.zeros((B, T, D), dtype=np.float32)
    for core in range(8):
        b = core // 2
        y[b] += res.results[core]["y"]
    if _trace:
        return y, res
    return y
